# revision 30
# baseline (speedup 1.0000x reference)
"""Trainium2 Bass kernel for nn_Net_24077586661451 (12-layer Mamba, d_model=70).

Sharding: 8 cores = 2 samples x 4 e-chunks (ED=140 -> 35/core).
Per-core scan grid: 560 partitions (35 e x 16 n, e-major p = e*16+n) as 5
partition tiles (4x128 + 48), L chunked by Q=512.

All matmuls run in bf16 (weights + moving operands; 0/1 selection matrices
are bf16-exact); PSUM accumulation stays fp32, as do the residual stream,
the scan state and the scan inputs (dA/dBx).

Layer pipeline (phases R/CS/X/G). The ACT instruction stream is phase-
ordered via per-instruction tile_wait_until stamps plus explicit
InstLoadActFuncSet loads of the natural_log_exp (rms/softplus/grid-exp)
and silu table sets, so each layer pays exactly two ~1.4us table loads
instead of thrashing on every ln<->exp<->silu transition. PE/DVE/DMA
instructions are unstamped and pipeline freely across phases and layers.
  R:  rmsnorm (ACT square + PE reduce + ACT ln/exp + PE bcast + DVE mul)
  CS: conv-fused in_proj (4 shifted-tap PE matmuls, norm_w/conv_w folded
      in), z-proj, silu (ACT)
  X:  stacked B|C x_proj (PE), softplus delta = ln(1+exp(.)) (ACT+DVE), u
  G:  per chunk: B/C broadcast once (shared by all 5 grid tiles since
      n(p) = p%16 everywhere) -> bf16 SBUF; per tile: PE seld broadcasts
      of delta/u -> PSUM, ACT exp(A_p * delta_b), DVE dBx mult (PSUM 1x),
      DVE tensor_tensor_scan (fp32 state), DVE hC mult (bf16 2x),
      PE n-reduction (+ diag(D_p)@xi folded into the accumulation);
      gate (DVE), AllGather y over the 4-core group (bf16, DRAM bounce),
      out_proj (PE), residual add (DVE, fp32).

Each core's xi channel order is permuted so its own 35 channels are rows 0:35
(weights permuted host-side; the program is identical across cores - SPMD).
"""
import ml_dtypes
import numpy as np

import concourse.bass as bass
import concourse.bacc as bacc
import concourse.mybir as mybir
import concourse.tile as tile
from concourse.bass_utils import run_bass_kernel_spmd

f32 = mybir.dt.float32
bf16 = mybir.dt.bfloat16
AF = mybir.ActivationFunctionType
OP = mybir.AluOpType

B, L, IN_DIM, D, ED, N, NL, DTR = 2, 2048, 32, 70, 140, 16, 12, 5
E = ED // 4                      # 35 channels per core
NCORES, GROUP = 8, 4
Q = 512
NCH = L // Q
EPS = 1e-5
# grid partition tiles: (pstart, pcount); p = e_loc*16 + n
GTILES = [(0, 128), (128, 128), (256, 128), (384, 128), (512, 48)]
# scan input/state tile dtype (scan state itself stays fp32 internally)
SCAN_DT = bf16

_CACHE = {}


def _consts():
    """Core-independent 0/1 selection matrices (bf16-exact).

    selbB/selbC broadcast the B rows (0:16) / C rows (16:32) of the stacked
    (32, Q) BsCs tile onto the grid; both contract over all 32 partitions so
    the moving operand keeps base partition 0.
    """
    seld = np.zeros((E, 5 * 128), np.float32)
    selbB = np.zeros((2 * N, 128), np.float32)
    selbC = np.zeros((2 * N, 128), np.float32)
    red = np.zeros((128, 5 * E), np.float32)
    for k, (pst, pc) in enumerate(GTILES):
        for p in range(pc):
            seld[8 * k + p // 16, k * 128 + p] = 1.0
            red[p, k * E + 8 * k + p // 16] = 1.0
    for p in range(128):
        selbB[p % 16, p] = 1.0
        selbC[N + p % 16, p] = 1.0
    return seld, selbB, selbC, red


def _build_nc(repeats=1):
    nc = bacc.Bacc("TRN2", target_bir_lowering=False, debug=False)

    di = {}  # dram inputs

    def dram_in(name, shape, dt=bf16):
        di[name] = nc.dram_tensor(name, list(shape), dt, kind="ExternalInput")
        return di[name]

    dram_in("x_t", (IN_DIM, L), f32)
    dram_in("w_in", (IN_DIM, D), f32)
    dram_in("b_in", (D, 1), f32)
    dram_in("taps", (D, NL * 4 * ED))
    dram_in("zw", (D, NL * E))
    dram_in("bcwA", (128, NL * 2 * N))
    dram_in("bcwB", (12, NL * 2 * N))
    dram_in("dtwA", (128, NL * E))
    dram_in("dtwB", (12, NL * E))
    dram_in("outwA", (128, NL * D))
    dram_in("outwB", (12, NL * D))
    dram_in("ddiag", (E, NL * E))
    dram_in("dtb", (E, NL), f32)
    dram_in("cbA", (128, NL), f32)
    dram_in("cbB", (12, NL), f32)
    dram_in("asc", (128, NL * 5), f32)
    dram_in("wout", (D, 1), f32)
    dram_in("bout", (1, 1), f32)
    dram_in("epsv", (128, 1), f32)
    out_d = nc.dram_tensor("out", [1, L], f32, kind="ExternalOutput")

    # inline constants (same on every core)
    seld_np, selbB_np, selbC_np, red_np = _consts()
    seld_d = nc.inline_tensor(seld_np.astype(ml_dtypes.bfloat16), "seld")
    selbB_d = nc.inline_tensor(selbB_np.astype(ml_dtypes.bfloat16), "selbB")
    selbC_d = nc.inline_tensor(selbC_np.astype(ml_dtypes.bfloat16), "selbC")
    red_d = nc.inline_tensor(red_np.astype(ml_dtypes.bfloat16), "red")
    ones70_d = nc.inline_tensor(np.ones((D, 1), ml_dtypes.bfloat16), "ones70")
    # rs broadcast selectors: rsel[:, c*D:(c+1)*D] has ones in row 32c —
    # broadcasts the packed per-chunk rms scale (partition 32c) to D rows.
    rsel_np = np.zeros((97, NCH * D), np.float32)
    for c in range(NCH):
        rsel_np[32 * c, c * D:(c + 1) * D] = 1.0
    rsel_d = nc.inline_tensor(rsel_np.astype(ml_dtypes.bfloat16), "rsel")

    with tile.TileContext(nc) as tc:
        with (
            tc.tile_pool(name="wts", bufs=1) as wts,
            tc.tile_pool(name="hbuf", bufs=1) as hbuf,
            tc.tile_pool(name="act", bufs=1) as act,         # per-layer activations
            tc.tile_pool(name="sb", bufs=4) as sb,           # per-chunk small sbuf
            tc.tile_pool(name="gsb", bufs=6) as gsb,         # grid sbuf (dA/dBx/hc)
            tc.tile_pool(name="hgr", bufs=3) as hgr,         # scan outputs (carry)
            tc.tile_pool(name="ps_b", bufs=4, space="PSUM") as ps_b,  # grid bcast
            tc.tile_pool(name="ps_xa", bufs=1, space="PSUM") as ps_xa,
            tc.tile_pool(name="ps_y", bufs=1, space="PSUM") as ps_y,
            tc.tile_pool(name="ps_s", bufs=2, space="PSUM") as ps_s,  # small psum
            tc.tile_pool(name="dr", bufs=2, space="DRAM") as dr,
        ):
            wt = {}
            for name, h in di.items():
                t = wts.tile(list(h.shape), h.dtype, tag=f"w_{name}")
                nc.sync.dma_start(t[:], h[:])
                wt[name] = t
            for name, h in (("seld", seld_d), ("selbB", selbB_d),
                            ("selbC", selbC_d), ("red", red_d),
                            ("ones70", ones70_d), ("rsel", rsel_d)):
                t = wts.tile(list(h.shape), h.dtype, tag=f"w_{name}")
                nc.sync.dma_start(t[:], h[:])
                wt[name] = t

            # persistent activation buffers
            h_a = hbuf.tile([D, L], f32)
            h_b = hbuf.tile([D, L], f32)
            hsc = hbuf.tile([D, L + 3], bf16)  # rms-scaled h, 3-col zero pad
            nc.vector.memset(hsc[:, 0:3], 0.0)

            # per-layer activation tiles
            xiA = act.tile([128, L], bf16)
            xiB = act.tile([12, L], bf16)
            zs = act.tile([E, L], bf16)
            BsCs = act.tile([2 * N, L], bf16)
            ez = act.tile([E, L], f32)
            ez1 = act.tile([E, L], f32)
            delta = act.tile([E, L], bf16)
            u = act.tile([E, L], bf16)

            # ---- embed: h_a = W_in @ x + b_in ----
            for c in range(NCH):
                sl = slice(c * Q, (c + 1) * Q)
                h0 = ps_s.tile([D, Q], f32, tag="psmall")
                nc.tensor.matmul(h0[:], wt["w_in"][:], wt["x_t"][:, sl])
                nc.scalar.activation(h_a[:, sl], h0[:], AF.Identity,
                                     bias=wt["b_in"][:, 0:1], scale=1.0)

            h_cur, h_nxt = h_a, h_b
            carry = [None] * 5  # previous chunk's h tiles (per grid tile)

            # natural_log_exp_and_others / silu_and_others table-set indices
            # in pwp act_info.json — explicit loads stop walrus's greedy
            # per-function choice (natural_log for ln, exp_and_others for
            # exp) from thrashing on every ln<->exp transition.
            SET_LNEXP, SET_SILU = 6, 18

            def load_act_set(set_id):
                nc.scalar.add_instruction(mybir.InstLoadActFuncSet(
                    name=nc.get_next_instruction_name(),
                    act_func_set_id=set_id, ins=[], outs=[]))

            def sact(stamp_ms, *args, **kwargs):
                # stamp ONLY the ACT instruction: orders the ACT stream by
                # phase (table-set sanity) without serializing PE/DVE/DMA
                # across phase or layer boundaries.
                with tc.tile_wait_until(stamp_ms):
                    nc.scalar.activation(*args, **kwargs)

            # phase stamps: strictly order each layer's four phases in the
            # Tile scheduler so silu-set ACT ops can't interleave with
            # ln/exp-set ones (each stray interleave costs two ~1.4us
            # ACT_TABLE_LOADs).
            ST = 10  # ms per phase slot; sim-time ordering only

            for li in range(NL * repeats):
                l = li % NL
                # ---- phase R: rmsnorm (ln/exp set) + conv matmuls ----
                # per-chunk mean-square packed at partitions {0,32,64,96} of
                # one PSUM tile -> a single Ln and Exp per layer.
                stR, stCS, stX, stG = ((4 * li + p) * ST for p in range(4))
                with tc.tile_wait_until(stR):
                    load_act_set(SET_LNEXP)
                msP = ps_s.tile([97, Q], f32, tag="psmall")
                nc.vector.memset(msP[:], 1.0)
                for c in range(NCH):
                    sl = slice(c * Q, (c + 1) * Q)
                    sq = sb.tile([D, Q], bf16, tag="sq")
                    sact(stR, sq[:], h_cur[:, sl], AF.Square)
                    nc.tensor.matmul(msP[32 * c:32 * c + 1, :],
                                     wt["ones70"][:], sq[:],
                                     tile_position=(0, 32 * c))
                lnv = sb.tile([97, Q], f32, tag="lnv")
                sact(stR, lnv[:], msP[:], AF.Ln,
                     bias=wt["epsv"][0:97, 0:1], scale=1.0 / D)
                rs = sb.tile([97, Q], bf16, tag="rs")
                sact(stR, rs[:], lnv[:], AF.Exp, scale=-0.5)
                for c in range(NCH):
                    sl = slice(c * Q, (c + 1) * Q)
                    rs70 = ps_s.tile([D, Q], f32, tag="psmall")
                    nc.tensor.matmul(rs70[:], wt["rsel"][:, c * D:(c + 1) * D],
                                     rs[:])
                    nc.vector.tensor_tensor(hsc[:, 3 + c * Q:3 + (c + 1) * Q],
                                            h_cur[:, sl], rs70[:], OP.mult)

                # ---- phase C+S: conv-fused in_proj, z proj, silu (silu set) ----
                with tc.tile_wait_until(stCS):
                    load_act_set(SET_SILU)
                for c in range(NCH):
                    xa = ps_xa.tile([128, Q], f32)
                    xb = ps_s.tile([12, Q], f32, tag="psmall")
                    for k in range(4):
                        tap = wt["taps"][:, (l * 4 + k) * ED:(l * 4 + k + 1) * ED]
                        rhs = hsc[:, c * Q + k:c * Q + k + Q]
                        nc.tensor.matmul(xa[:], tap[:, 0:128], rhs,
                                         start=(k == 0), stop=(k == 3))
                        nc.tensor.matmul(xb[:], tap[:, 128:ED], rhs,
                                         start=(k == 0), stop=(k == 3))
                    sl = slice(c * Q, (c + 1) * Q)
                    sact(stCS, xiA[:, sl], xa[:], AF.Silu,
                         bias=wt["cbA"][:, l:l + 1], scale=1.0)
                    sact(stCS, xiB[:, sl], xb[:], AF.Silu,
                         bias=wt["cbB"][:, l:l + 1], scale=1.0)
                    zp = ps_s.tile([E, Q], f32, tag="psmall")
                    nc.tensor.matmul(zp[:], wt["zw"][:, l * E:(l + 1) * E],
                                     hsc[:, 3 + c * Q:3 + (c + 1) * Q])
                    sact(stCS, zs[:, sl], zp[:], AF.Silu)

                # ---- phase X: x_proj B|C, dt -> softplus (ln/exp set), u ----
                with tc.tile_wait_until(stX):
                    load_act_set(SET_LNEXP)
                for c in range(NCH):
                    sl = slice(c * Q, (c + 1) * Q)
                    bc = ps_s.tile([2 * N, Q], f32, tag="psmall")
                    nc.tensor.matmul(bc[:],
                                     wt["bcwA"][:, l * 2 * N:(l + 1) * 2 * N],
                                     xiA[:, sl], start=True, stop=False)
                    nc.tensor.matmul(bc[:],
                                     wt["bcwB"][:, l * 2 * N:(l + 1) * 2 * N],
                                     xiB[:, sl], start=False, stop=True)
                    dpre = ps_s.tile([E, Q], f32, tag="psmall")
                    nc.tensor.matmul(dpre[:], wt["dtwA"][:, l * E:(l + 1) * E],
                                     xiA[:, sl], start=True, stop=False)
                    nc.tensor.matmul(dpre[:], wt["dtwB"][:, l * E:(l + 1) * E],
                                     xiB[:, sl], start=False, stop=True)
                    sact(stX, ez[:, sl], dpre[:], AF.Exp,
                         bias=wt["dtb"][:, l:l + 1], scale=1.0)
                    sact(stX, BsCs[:, sl], bc[:], AF.Copy)
                nc.vector.tensor_scalar_add(ez1[:], ez[:], 1.0)
                sact(stX, delta[:], ez1[:], AF.Ln)
                nc.vector.tensor_tensor(u[:], delta[:], xiA[0:E, :], OP.mult)

                # ---- phase G: scan grid ----
                for c in range(NCH):
                    sl = slice(c * Q, (c + 1) * Q)
                    y_ps = ps_y.tile([E, Q], f32, tag="ypsum")
                    # D_p * xi folded into the n-reduction accumulation
                    nc.tensor.matmul(y_ps[:], wt["ddiag"][:, l * E:(l + 1) * E],
                                     xiA[0:E, sl], start=True, stop=False)
                    # B/C broadcasts are identical for all 5 grid tiles
                    # (n(p) = p%16 everywhere): compute once per chunk.
                    Bbp = ps_b.tile([128, Q], f32, tag="bc")
                    nc.tensor.matmul(Bbp[:], wt["selbB"][:], BsCs[:, sl])
                    Bbg = gsb.tile([128, Q], bf16, tag="Bbg")
                    sact(stG, Bbg[:], Bbp[:], AF.Copy)
                    Cbp = ps_b.tile([128, Q], f32, tag="bc")
                    nc.tensor.matmul(Cbp[:], wt["selbC"][:], BsCs[:, sl])
                    Cbg = gsb.tile([128, Q], bf16, tag="Cbg")
                    sact(stG, Cbg[:], Cbp[:], AF.Copy)
                    new_carry = [None] * 5
                    for k, (pst, pc) in enumerate(GTILES):
                        sd = wt["seld"][:, k * 128:k * 128 + pc]
                        db = ps_b.tile([128, Q], f32, tag="bc")
                        nc.tensor.matmul(db[0:pc, :], sd, delta[:, sl])
                        dA = gsb.tile([128, Q], SCAN_DT, tag="dA")
                        with tc.tile_wait_until(stG):
                            nc.scalar.activation(
                                dA[0:pc, :], db[0:pc, :], AF.Exp,
                                scale=wt["asc"][0:pc, l * 5 + k:l * 5 + k + 1])
                        ub = ps_b.tile([128, Q], f32, tag="bc")
                        nc.tensor.matmul(ub[0:pc, :], sd, u[:, sl])
                        dBx = gsb.tile([128, Q], SCAN_DT, tag="dBx")
                        nc.vector.tensor_tensor(dBx[0:pc, :], ub[0:pc, :],
                                                Bbg[0:pc, :], OP.mult)
                        hgt = hgr.tile([128, Q], SCAN_DT, tag=f"h{k}")
                        init = 0.0 if c == 0 else carry[k][0:pc, Q - 1:Q]
                        nc.vector.tensor_tensor_scan(
                            hgt[0:pc, :], dA[0:pc, :], dBx[0:pc, :], init,
                            OP.mult, OP.add)
                        new_carry[k] = hgt
                        hc = gsb.tile([128, Q], bf16, tag="hc")
                        nc.vector.tensor_tensor(hc[0:pc, :], hgt[0:pc, :],
                                                Cbg[0:pc, :], OP.mult)
                        nc.tensor.matmul(y_ps[:], wt["red"][0:pc, k * E:(k + 1) * E],
                                         hc[0:pc, :], start=False, stop=(k == 4))
                    carry = new_carry

                    # ---- gate ----
                    yg = sb.tile([E, Q], bf16, tag="yg")
                    nc.vector.tensor_tensor(yg[:], y_ps[:], zs[:, sl], OP.mult)

                    # ---- all-gather y over the 4-core group ----
                    ygd = dr.tile([E, Q], bf16, tag="ygd")
                    nc.sync.dma_start(ygd[:], yg[:])
                    yga = dr.tile([GROUP * E, Q], bf16, tag="yga")
                    nc.gpsimd.collective_compute(
                        "AllGather", OP.bypass,
                        replica_groups=[[0, 1, 2, 3], [4, 5, 6, 7]],
                        ins=[ygd.opt()], outs=[yga.opt()])
                    yfA = sb.tile([128, Q], bf16, tag="yfA")
                    yfB = sb.tile([12, Q], bf16, tag="yfB")
                    nc.sync.dma_start(yfA[:], yga[0:128, :])
                    nc.sync.dma_start(yfB[:], yga[128:ED, :])

                    # ---- out_proj + residual ----
                    op_ps = ps_s.tile([D, Q], f32, tag="psmall")
                    nc.tensor.matmul(op_ps[:], wt["outwA"][:, l * D:(l + 1) * D],
                                     yfA[:], start=True, stop=False)
                    nc.tensor.matmul(op_ps[:], wt["outwB"][:, l * D:(l + 1) * D],
                                     yfB[:], start=False, stop=True)
                    nc.vector.tensor_tensor(h_nxt[:, sl], h_cur[:, sl], op_ps[:],
                                            OP.add)
                h_cur, h_nxt = h_nxt, h_cur

            # ---- head ----
            for c in range(NCH):
                sl = slice(c * Q, (c + 1) * Q)
                hp = ps_s.tile([1, Q], f32, tag="psmall")
                nc.tensor.matmul(hp[:], wt["wout"][:], h_cur[:, sl])
                ot = sb.tile([1, Q], f32, tag="ot")
                nc.scalar.activation(ot[:], hp[:], AF.Tanh,
                                     bias=wt["bout"][:, 0:1], scale=1.0)
                nc.sync.dma_start(out_d[:, sl], ot[:])

    nc.compile()
    return nc


def _prep_inputs(inputs):
    """Returns in_maps: list of 8 dicts (core = s*4 + j)."""
    g = {k: np.asarray(v, np.float32) for k, v in inputs.items()}
    nw, ipw = g["norm_w"], g["in_proj_w"]
    cw, cb = g["conv_w"], g["conv_b"]
    xpw, dtw, dtb = g["x_proj_w"], g["dt_w"], g["dt_b"]
    alog, dpv, opw = g["A_log"], g["D_p"], g["out_proj_w"]

    def b16(x):
        return np.ascontiguousarray(x.astype(ml_dtypes.bfloat16))

    maps = []
    for s in range(2):
        for j in range(4):
            own = np.arange(E * j, E * (j + 1))
            perm = np.r_[own, np.delete(np.arange(ED), own)]
            m = {
                "x_t": np.ascontiguousarray(g["x"][s].T),
                "w_in": np.ascontiguousarray(g["W_in"].T),
                "b_in": g["b_in"].reshape(D, 1),
                "dtb": np.stack([dtb[l][own] for l in range(NL)], 1),
                "wout": np.ascontiguousarray(g["W_out"].T),
                "bout": g["b_out"].reshape(1, 1),
                "epsv": np.full((128, 1), EPS, np.float32),
            }
            taps = np.zeros((D, NL * 4 * ED), np.float32)
            zw = np.zeros((D, NL * E), np.float32)
            bcw = np.zeros((ED, NL * 2 * N), np.float32)
            dtwT = np.zeros((ED, NL * E), np.float32)
            outw = np.zeros((ED, NL * D), np.float32)
            ddiag = np.zeros((E, NL * E), np.float32)
            cbp = np.zeros((ED, NL), np.float32)
            asc = np.zeros((128, NL * 5), np.float32)
            for l in range(NL):
                Wxi = ipw[l][:ED] * nw[l][None, :]          # (140,70)
                for k in range(4):
                    tap = (cw[l, :, 0, k:k + 1] * Wxi)[perm]
                    taps[:, (l * 4 + k) * ED:(l * 4 + k + 1) * ED] = tap.T
                zw[:, l * E:(l + 1) * E] = (ipw[l][ED:2 * ED] * nw[l][None, :])[own].T
                bcw[:, l * 2 * N:(l + 1) * 2 * N] = \
                    xpw[l][DTR:DTR + 2 * N][:, perm].T
                mdt = dtw[l][own] @ xpw[l][0:DTR]           # (35,140)
                dtwT[:, l * E:(l + 1) * E] = mdt[:, perm].T
                outw[:, l * D:(l + 1) * D] = opw[l].T
                ddiag[:, l * E:(l + 1) * E] = np.diag(dpv[l][own])
                cbp[:, l] = cb[l][perm]
                A = -np.exp(alog[l])                        # (140,16)
                Ao = A[own]                                 # (35,16)
                for k, (pst, pc) in enumerate(GTILES):
                    e0 = 8 * k
                    v = Ao[e0:e0 + pc // 16].reshape(-1)    # (pc,)
                    asc[0:pc, l * 5 + k] = v
            m.update(taps=b16(taps), zw=b16(zw),
                     bcwA=b16(bcw[0:128]), bcwB=b16(bcw[128:ED]),
                     dtwA=b16(dtwT[0:128]), dtwB=b16(dtwT[128:ED]),
                     outwA=b16(outw[0:128]), outwB=b16(outw[128:ED]),
                     ddiag=b16(ddiag),
                     cbA=cbp[0:128], cbB=cbp[128:ED], asc=asc)
            maps.append(m)
    return maps


def kernel(**inputs):
    if "nc" not in _CACHE:
        _CACHE["nc"] = _build_nc()
    nc = _CACHE["nc"]
    in_maps = _prep_inputs(inputs)
    res = run_bass_kernel_spmd(nc, in_maps, list(range(NCORES))).results
    out = np.concatenate([res[0]["out"].ravel(), res[4]["out"].ravel()])
    return out.astype(np.float32)


# revision 34
# speedup vs baseline: 1.0549x; 1.0549x over previous
"""Trainium2 Bass kernel for nn_Net_24077586661451 (12-layer Mamba, d_model=70).

Sharding: 8 cores = 2 samples x 4 e-chunks (ED=140 -> 35/core).
Per-core scan grid: 560 partitions (35 e x 16 n, e-major p = e*16+n) as 5
partition tiles (4x128 + 48), L chunked by Q=512.

All matmuls run in bf16 (weights + moving operands; 0/1 selection matrices
are bf16-exact); PSUM accumulation stays fp32, as do the residual stream,
the scan state and the scan inputs (dA/dBx).

Layer pipeline (phases R/CS/X/G). The ACT instruction stream is phase-
ordered via per-instruction tile_wait_until stamps plus explicit
InstLoadActFuncSet loads of the natural_log_exp (rms/softplus/grid-exp)
and silu table sets, so each layer pays exactly two ~1.4us table loads
instead of thrashing on every ln<->exp<->silu transition. PE/DVE/DMA
instructions are unstamped and pipeline freely across phases and layers.
  R:  rmsnorm (ACT square + PE reduce + ACT ln/exp + PE bcast + DVE mul)
  CS: conv-fused in_proj (4 shifted-tap PE matmuls, norm_w/conv_w folded
      in), z-proj, silu (ACT)
  X:  stacked B|C x_proj (PE), softplus delta = ln(1+exp(.)) (ACT+DVE), u
  G:  per chunk: B/C broadcast once (shared by all 5 grid tiles since
      n(p) = p%16 everywhere) -> bf16 SBUF; per tile: PE seld broadcasts
      of delta/u -> PSUM, ACT exp(A_p * delta_b), DVE dBx mult (PSUM 1x),
      DVE tensor_tensor_scan (fp32 state), DVE hC mult (bf16 2x),
      PE n-reduction (+ diag(D_p)@xi folded into the accumulation);
      gate (DVE), AllGather y over the 4-core group (bf16, DRAM bounce),
      out_proj (PE), residual add (DVE, fp32).

Each core's xi channel order is permuted so its own 35 channels are rows 0:35
(weights permuted host-side; the program is identical across cores - SPMD).
"""
import ml_dtypes
import numpy as np

import concourse.bass as bass
import concourse.bass_isa as bass_isa
import concourse.bacc as bacc
import concourse.mybir as mybir
import concourse.tile as tile
from concourse.bass_utils import run_bass_kernel_spmd

f32 = mybir.dt.float32
bf16 = mybir.dt.bfloat16
AF = mybir.ActivationFunctionType
OP = mybir.AluOpType

B, L, IN_DIM, D, ED, N, NL, DTR = 2, 2048, 32, 70, 140, 16, 12, 5
E = ED // 4                      # 35 channels per core
NCORES, GROUP = 8, 4
Q = 512
NCH = L // Q
EPS = 1e-5
# grid partition tiles: (pstart, pcount); p = e_loc*16 + n
GTILES = [(0, 128), (128, 128), (256, 128), (384, 128), (512, 48)]
# scan input/state tile dtype (scan state itself stays fp32 internally)
SCAN_DT = bf16

_CACHE = {}


def _consts():
    """Core-independent 0/1 selection matrices (bf16-exact).

    selbB/selbC broadcast the B rows (0:16) / C rows (16:32) of the stacked
    (32, Q) BsCs tile onto the grid; both contract over all 32 partitions so
    the moving operand keeps base partition 0.
    """
    seld = np.zeros((E, 5 * 128), np.float32)
    selbB = np.zeros((96, 128), np.float32)
    selbC = np.zeros((96, 128), np.float32)
    red = np.zeros((128, 5 * E), np.float32)
    for k, (pst, pc) in enumerate(GTILES):
        for p in range(pc):
            seld[8 * k + p // 16, k * 128 + p] = 1.0
            red[p, k * E + 8 * k + p // 16] = 1.0
    for p in range(128):
        selbB[64 + p % 16, p] = 1.0
        selbC[64 + N + p % 16, p] = 1.0
    return seld, selbB, selbC, red


def _build_nc(repeats=1):
    nc = bacc.Bacc("TRN2", target_bir_lowering=False, debug=False)

    di = {}  # dram inputs

    def dram_in(name, shape, dt=bf16):
        di[name] = nc.dram_tensor(name, list(shape), dt, kind="ExternalInput")
        return di[name]

    dram_in("x_t", (IN_DIM, L), f32)
    dram_in("w_in", (IN_DIM, D), f32)
    dram_in("b_in", (D, 1), f32)
    dram_in("taps", (D, NL * 4 * ED))
    dram_in("zw", (D, NL * E))
    dram_in("bcdpA", (128, NL * 96))
    dram_in("bcdpB", (12, NL * 96))
    dram_in("outwA", (128, NL * D))
    dram_in("outwB", (12, NL * D))
    dram_in("ddiag", (E, NL * E))
    dram_in("dtb", (E, NL), f32)
    dram_in("cbA", (128, NL), f32)
    dram_in("cbB", (12, NL), f32)
    dram_in("asc", (128, NL * 5), f32)
    dram_in("wout", (D, 1), f32)
    dram_in("bout", (1, 1), f32)
    dram_in("epsv", (128, 1), f32)
    out_d = nc.dram_tensor("out", [1, L], f32, kind="ExternalOutput")

    # inline constants (same on every core)
    seld_np, selbB_np, selbC_np, red_np = _consts()
    seld_d = nc.inline_tensor(seld_np.astype(ml_dtypes.bfloat16), "seld")
    selbB_d = nc.inline_tensor(selbB_np.astype(ml_dtypes.bfloat16), "selbB")
    selbC_d = nc.inline_tensor(selbC_np.astype(ml_dtypes.bfloat16), "selbC")
    red_d = nc.inline_tensor(red_np.astype(ml_dtypes.bfloat16), "red")
    ones70_d = nc.inline_tensor(np.ones((D, 1), ml_dtypes.bfloat16), "ones70")
    # rs broadcast selectors: rsel[:, c*D:(c+1)*D] has ones in row 32c —
    # broadcasts the packed per-chunk rms scale (partition 32c) to D rows.
    rsel_np = np.zeros((97, NCH * D), np.float32)
    for c in range(NCH):
        rsel_np[32 * c, c * D:(c + 1) * D] = 1.0
    rsel_d = nc.inline_tensor(rsel_np.astype(ml_dtypes.bfloat16), "rsel")

    with tile.TileContext(nc) as tc:
        with (
            tc.tile_pool(name="wts", bufs=1) as wts,
            tc.tile_pool(name="hbuf", bufs=1) as hbuf,
            tc.tile_pool(name="act", bufs=1) as act,         # per-layer activations
            tc.tile_pool(name="sb", bufs=4) as sb,           # per-chunk small sbuf
            tc.tile_pool(name="gsb", bufs=6) as gsb,         # grid sbuf (dA/dBx/hc)
            tc.tile_pool(name="hgr", bufs=3) as hgr,         # scan outputs (carry)
            tc.tile_pool(name="ps_b", bufs=4, space="PSUM") as ps_b,  # grid bcast
            tc.tile_pool(name="ps_xa", bufs=1, space="PSUM") as ps_xa,
            tc.tile_pool(name="ps_y", bufs=1, space="PSUM") as ps_y,
            tc.tile_pool(name="ps_s", bufs=2, space="PSUM") as ps_s,  # small psum
            tc.tile_pool(name="dr", bufs=2, space="DRAM") as dr,
        ):
            wt = {}
            for name, h in di.items():
                t = wts.tile(list(h.shape), h.dtype, tag=f"w_{name}")
                nc.sync.dma_start(t[:], h[:])
                wt[name] = t
            for name, h in (("seld", seld_d), ("selbB", selbB_d),
                            ("selbC", selbC_d), ("red", red_d),
                            ("ones70", ones70_d), ("rsel", rsel_d)):
                t = wts.tile(list(h.shape), h.dtype, tag=f"w_{name}")
                nc.sync.dma_start(t[:], h[:])
                wt[name] = t

            # persistent activation buffers
            h_a = hbuf.tile([D, L], f32)
            h_b = hbuf.tile([D, L], f32)
            hsc = hbuf.tile([D, L + 3], bf16)  # rms-scaled h, 3-col zero pad
            nc.vector.memset(hsc[:, 0:3], 0.0)

            # per-layer activation tiles
            xiA = act.tile([128, L], bf16)
            xiB = act.tile([12, L], bf16)
            zs = act.tile([E, L], bf16)
            BsCs = act.tile([96, L], bf16)
            # rows 0:64 are never written but the B/C broadcast matmuls
            # contract over all 96 partitions (0-weighted) — zero them so
            # stale NaNs can't poison 0*x products.
            nc.vector.memset(BsCs[0:64, :], 0.0)
            ez = act.tile([E, L], f32)
            ez1 = act.tile([E, L], f32)
            delta = act.tile([E, L], bf16)
            u = act.tile([E, L], bf16)

            # ---- embed: h_a = W_in @ x + b_in ----
            for c in range(NCH):
                sl = slice(c * Q, (c + 1) * Q)
                h0 = ps_s.tile([D, Q], f32, tag="psmall")
                nc.tensor.matmul(h0[:], wt["w_in"][:], wt["x_t"][:, sl])
                nc.scalar.activation(h_a[:, sl], h0[:], AF.Identity,
                                     bias=wt["b_in"][:, 0:1], scale=1.0)

            h_cur, h_nxt = h_a, h_b
            carry = [None] * 5  # previous chunk's h tiles (per grid tile)

            # natural_log_exp_and_others / silu_and_others table-set indices
            # in pwp act_info.json — explicit loads stop walrus's greedy
            # per-function choice (natural_log for ln, exp_and_others for
            # exp) from thrashing on every ln<->exp transition.
            SET_LNEXP, SET_SILU = 6, 18

            def load_act_set(set_id):
                nc.scalar.add_instruction(mybir.InstLoadActFuncSet(
                    name=nc.get_next_instruction_name(),
                    act_func_set_id=set_id, ins=[], outs=[]))

            def sact(stamp_ms, *args, **kwargs):
                # stamp ONLY the ACT instruction: orders the ACT stream by
                # phase (table-set sanity) without serializing PE/DVE/DMA
                # across phase or layer boundaries.
                with tc.tile_wait_until(stamp_ms):
                    nc.scalar.activation(*args, **kwargs)

            # phase stamps: strictly order each layer's four phases in the
            # Tile scheduler so silu-set ACT ops can't interleave with
            # ln/exp-set ones (each stray interleave costs two ~1.4us
            # ACT_TABLE_LOADs).
            ST = 10  # ms per phase slot; sim-time ordering only

            for li in range(NL * repeats):
                l = li % NL
                # ---- phase R: rmsnorm (ln/exp set) + conv matmuls ----
                # per-chunk mean-square packed at partitions {0,32,64,96} of
                # one PSUM tile -> a single Ln and Exp per layer.
                stR, stCS, stX, stG = ((4 * li + p) * ST for p in range(4))
                with tc.tile_wait_until(stR):
                    load_act_set(SET_LNEXP)
                for c in range(NCH):
                    sl = slice(c * Q, (c + 1) * Q)
                    sq = sb.tile([D, Q], f32, tag="sq")
                    sact(stR, sq[:], h_cur[:, sl], AF.Square)
                    # sum over the 70 model dims on GpSimd; result arrives
                    # replicated to all 70 partitions, so ln/exp run on the
                    # full tile (same FD-bound cost) and no PE broadcast of
                    # the rsqrt scale is needed.
                    vms = sb.tile([D, Q], f32, tag="vms")
                    nc.gpsimd.partition_all_reduce(vms[:], sq[:], D,
                                                   bass_isa.ReduceOp.add)
                    lnv = sb.tile([D, Q], f32, tag="lnv")
                    sact(stR, lnv[:], vms[:], AF.Ln,
                         bias=wt["epsv"][0:D, 0:1], scale=1.0 / D)
                    rs70 = sb.tile([D, Q], bf16, tag="rs")
                    sact(stR, rs70[:], lnv[:], AF.Exp, scale=-0.5)
                    nc.vector.tensor_tensor(hsc[:, 3 + c * Q:3 + (c + 1) * Q],
                                            h_cur[:, sl], rs70[:], OP.mult)

                # ---- phase C+S: conv-fused in_proj, z proj, silu (silu set) ----
                with tc.tile_wait_until(stCS):
                    load_act_set(SET_SILU)
                for c in range(NCH):
                    xa = ps_xa.tile([128, Q], f32)
                    xb = ps_s.tile([12, Q], f32, tag="psmall")
                    for k in range(4):
                        tap = wt["taps"][:, (l * 4 + k) * ED:(l * 4 + k + 1) * ED]
                        rhs = hsc[:, c * Q + k:c * Q + k + Q]
                        nc.tensor.matmul(xa[:], tap[:, 0:128], rhs,
                                         start=(k == 0), stop=(k == 3))
                        nc.tensor.matmul(xb[:], tap[:, 128:ED], rhs,
                                         start=(k == 0), stop=(k == 3))
                    sl = slice(c * Q, (c + 1) * Q)
                    sact(stCS, xiA[:, sl], xa[:], AF.Silu,
                         bias=wt["cbA"][:, l:l + 1], scale=1.0)
                    sact(stCS, xiB[:, sl], xb[:], AF.Silu,
                         bias=wt["cbB"][:, l:l + 1], scale=1.0)
                    zp = ps_s.tile([E, Q], f32, tag="psmall")
                    nc.tensor.matmul(zp[:], wt["zw"][:, l * E:(l + 1) * E],
                                     hsc[:, 3 + c * Q:3 + (c + 1) * Q])
                    sact(stCS, zs[:, sl], zp[:], AF.Silu)

                # ---- phase X: x_proj B|C, dt -> softplus (ln/exp set), u ----
                with tc.tile_wait_until(stX):
                    load_act_set(SET_LNEXP)
                W = 96
                for c in range(NCH):
                    sl = slice(c * Q, (c + 1) * Q)
                    bcdp = ps_s.tile([W, Q], f32, tag="psmall")
                    nc.tensor.matmul(bcdp[:], wt["bcdpA"][:, l * W:(l + 1) * W],
                                     xiA[:, sl], start=True, stop=False)
                    nc.tensor.matmul(bcdp[:], wt["bcdpB"][:, l * W:(l + 1) * W],
                                     xiB[:, sl], start=False, stop=True)
                    sact(stX, ez[:, sl], bcdp[0:E, :], AF.Exp,
                         bias=wt["dtb"][:, l:l + 1], scale=1.0)
                    sact(stX, BsCs[64:W, sl], bcdp[64:W, :], AF.Copy)
                nc.vector.tensor_scalar_add(ez1[:], ez[:], 1.0)
                sact(stX, delta[:], ez1[:], AF.Ln)
                nc.vector.tensor_tensor(u[:], delta[:], xiA[0:E, :], OP.mult)

                # ---- phase G: scan grid ----
                for c in range(NCH):
                    sl = slice(c * Q, (c + 1) * Q)
                    y_ps = ps_y.tile([E, Q], f32, tag="ypsum")
                    # D_p * xi folded into the n-reduction accumulation
                    nc.tensor.matmul(y_ps[:], wt["ddiag"][:, l * E:(l + 1) * E],
                                     xiA[0:E, sl], start=True, stop=False)
                    # B/C broadcasts are identical for all 5 grid tiles
                    # (n(p) = p%16 everywhere): compute once per chunk.
                    Bbp = ps_b.tile([128, Q], f32, tag="bc")
                    nc.tensor.matmul(Bbp[:], wt["selbB"][:], BsCs[:, sl])
                    Bbg = gsb.tile([128, Q], bf16, tag="Bbg")
                    sact(stG, Bbg[:], Bbp[:], AF.Copy)
                    Cbp = ps_b.tile([128, Q], f32, tag="bc")
                    nc.tensor.matmul(Cbp[:], wt["selbC"][:], BsCs[:, sl])
                    Cbg = gsb.tile([128, Q], bf16, tag="Cbg")
                    sact(stG, Cbg[:], Cbp[:], AF.Copy)
                    new_carry = [None] * 5
                    for k, (pst, pc) in enumerate(GTILES):
                        sd = wt["seld"][:, k * 128:k * 128 + pc]
                        db = ps_b.tile([128, Q], f32, tag="bc")
                        nc.tensor.matmul(db[0:pc, :], sd, delta[:, sl])
                        dA = gsb.tile([128, Q], SCAN_DT, tag="dA")
                        with tc.tile_wait_until(stG):
                            nc.scalar.activation(
                                dA[0:pc, :], db[0:pc, :], AF.Exp,
                                scale=wt["asc"][0:pc, l * 5 + k:l * 5 + k + 1])
                        ub = ps_b.tile([128, Q], f32, tag="bc")
                        nc.tensor.matmul(ub[0:pc, :], sd, u[:, sl])
                        dBx = gsb.tile([128, Q], SCAN_DT, tag="dBx")
                        nc.vector.tensor_tensor(dBx[0:pc, :], ub[0:pc, :],
                                                Bbg[0:pc, :], OP.mult)
                        hgt = hgr.tile([128, Q], SCAN_DT, tag=f"h{k}")
                        init = 0.0 if c == 0 else carry[k][0:pc, Q - 1:Q]
                        nc.vector.tensor_tensor_scan(
                            hgt[0:pc, :], dA[0:pc, :], dBx[0:pc, :], init,
                            OP.mult, OP.add)
                        new_carry[k] = hgt
                        hc = gsb.tile([128, Q], bf16, tag="hc")
                        nc.vector.tensor_tensor(hc[0:pc, :], hgt[0:pc, :],
                                                Cbg[0:pc, :], OP.mult)
                        nc.tensor.matmul(y_ps[:], wt["red"][0:pc, k * E:(k + 1) * E],
                                         hc[0:pc, :], start=False, stop=(k == 4))
                    carry = new_carry

                    # ---- gate ----
                    yg = sb.tile([E, Q], bf16, tag="yg")
                    nc.vector.tensor_tensor(yg[:], y_ps[:], zs[:, sl], OP.mult)

                    # ---- all-gather y over the 4-core group ----
                    ygd = dr.tile([E, Q], bf16, tag="ygd")
                    nc.sync.dma_start(ygd[:], yg[:])
                    yga = dr.tile([GROUP * E, Q], bf16, tag="yga")
                    nc.gpsimd.collective_compute(
                        "AllGather", OP.bypass,
                        replica_groups=[[0, 1, 2, 3], [4, 5, 6, 7]],
                        ins=[ygd.opt()], outs=[yga.opt()])
                    yfA = sb.tile([128, Q], bf16, tag="yfA")
                    yfB = sb.tile([12, Q], bf16, tag="yfB")
                    nc.sync.dma_start(yfA[:], yga[0:128, :])
                    nc.sync.dma_start(yfB[:], yga[128:ED, :])

                    # ---- out_proj + residual ----
                    op_ps = ps_s.tile([D, Q], f32, tag="psmall")
                    nc.tensor.matmul(op_ps[:], wt["outwA"][:, l * D:(l + 1) * D],
                                     yfA[:], start=True, stop=False)
                    nc.tensor.matmul(op_ps[:], wt["outwB"][:, l * D:(l + 1) * D],
                                     yfB[:], start=False, stop=True)
                    nc.vector.tensor_tensor(h_nxt[:, sl], h_cur[:, sl], op_ps[:],
                                            OP.add)
                h_cur, h_nxt = h_nxt, h_cur

            # ---- head ----
            for c in range(NCH):
                sl = slice(c * Q, (c + 1) * Q)
                hp = ps_s.tile([1, Q], f32, tag="psmall")
                nc.tensor.matmul(hp[:], wt["wout"][:], h_cur[:, sl])
                ot = sb.tile([1, Q], f32, tag="ot")
                nc.scalar.activation(ot[:], hp[:], AF.Tanh,
                                     bias=wt["bout"][:, 0:1], scale=1.0)
                nc.sync.dma_start(out_d[:, sl], ot[:])

    nc.compile()
    return nc


def _prep_inputs(inputs):
    """Returns in_maps: list of 8 dicts (core = s*4 + j)."""
    g = {k: np.asarray(v, np.float32) for k, v in inputs.items()}
    nw, ipw = g["norm_w"], g["in_proj_w"]
    cw, cb = g["conv_w"], g["conv_b"]
    xpw, dtw, dtb = g["x_proj_w"], g["dt_w"], g["dt_b"]
    alog, dpv, opw = g["A_log"], g["D_p"], g["out_proj_w"]

    def b16(x):
        return np.ascontiguousarray(x.astype(ml_dtypes.bfloat16))

    maps = []
    for s in range(2):
        for j in range(4):
            own = np.arange(E * j, E * (j + 1))
            perm = np.r_[own, np.delete(np.arange(ED), own)]
            m = {
                "x_t": np.ascontiguousarray(g["x"][s].T),
                "w_in": np.ascontiguousarray(g["W_in"].T),
                "b_in": g["b_in"].reshape(D, 1),
                "dtb": np.stack([dtb[l][own] for l in range(NL)], 1),
                "wout": np.ascontiguousarray(g["W_out"].T),
                "bout": g["b_out"].reshape(1, 1),
                "epsv": np.full((128, 1), EPS, np.float32),
            }
            taps = np.zeros((D, NL * 4 * ED), np.float32)
            zw = np.zeros((D, NL * E), np.float32)
            Wst = 96
            bcdp = np.zeros((ED, NL * Wst), np.float32)
            outw = np.zeros((ED, NL * D), np.float32)
            ddiag = np.zeros((E, NL * E), np.float32)
            cbp = np.zeros((ED, NL), np.float32)
            asc = np.zeros((128, NL * 5), np.float32)
            for l in range(NL):
                Wxi = ipw[l][:ED] * nw[l][None, :]          # (140,70)
                for k in range(4):
                    tap = (cw[l, :, 0, k:k + 1] * Wxi)[perm]
                    taps[:, (l * 4 + k) * ED:(l * 4 + k + 1) * ED] = tap.T
                zw[:, l * E:(l + 1) * E] = (ipw[l][ED:2 * ED] * nw[l][None, :])[own].T
                mdt = dtw[l][own] @ xpw[l][0:DTR]           # (35,140)
                bcdp[:, l * Wst:l * Wst + E] = mdt[:, perm].T
                bcdp[:, l * Wst + 64:(l + 1) * Wst] = \
                    xpw[l][DTR:DTR + 2 * N][:, perm].T
                outw[:, l * D:(l + 1) * D] = opw[l].T
                ddiag[:, l * E:(l + 1) * E] = np.diag(dpv[l][own])
                cbp[:, l] = cb[l][perm]
                A = -np.exp(alog[l])                        # (140,16)
                Ao = A[own]                                 # (35,16)
                for k, (pst, pc) in enumerate(GTILES):
                    e0 = 8 * k
                    v = Ao[e0:e0 + pc // 16].reshape(-1)    # (pc,)
                    asc[0:pc, l * 5 + k] = v
            m.update(taps=b16(taps), zw=b16(zw),
                     bcdpA=b16(bcdp[0:128]), bcdpB=b16(bcdp[128:ED]),
                     outwA=b16(outw[0:128]), outwB=b16(outw[128:ED]),
                     ddiag=b16(ddiag),
                     cbA=cbp[0:128], cbB=cbp[128:ED], asc=asc)
            maps.append(m)
    return maps


def kernel(**inputs):
    if "nc" not in _CACHE:
        _CACHE["nc"] = _build_nc()
    nc = _CACHE["nc"]
    in_maps = _prep_inputs(inputs)
    res = run_bass_kernel_spmd(nc, in_maps, list(range(NCORES))).results
    out = np.concatenate([res[0]["out"].ravel(), res[4]["out"].ravel()])
    return out.astype(np.float32)
